# revision 9
# baseline (speedup 1.0000x reference)
"""Trainium2 Bass kernel for nn_Classifier_71829033059182 (embedding_lookup).

Data-parallel across 8 NeuronCores: batch is sharded (512 rows/core); the
type LUT, frequencies and MLP weights are replicated. Per core:
  - chunked dma_gather (1024 idxs/call) of an augmented fp32 LUT row
    [v(128) | freq | pad] for feature math (batch-major layout), and a
    transposed bf16 gather of the LUT for the layer-1 matmul (feature-major),
  - Poincare/cos/dot/popularity features on DVE+ACT,
  - 4-layer MLP on the PE (bf16 for the v-block of W1, fp32r elsewhere),
  - sigmoid distribution + BCE-loss partials; host sums partials.
"""
import numpy as np
import ml_dtypes
from contextlib import ExitStack

import concourse.bass as bass
import concourse.tile as tile
import concourse.mybir as mybir
from concourse import bacc
from concourse.bass_utils import run_bass_kernel_spmd

F32 = mybir.dt.float32
F32R = mybir.dt.float32r
BF16 = mybir.dt.bfloat16
I16 = mybir.dt.int16
AF = mybir.ActivationFunctionType
ALU = mybir.AluOpType
AX = mybir.AxisListType

B, K, D, H, V = 4096, 64, 128, 2048, 10000
EPS = 1e-5
NCORES = 8
AUG = 192           # augmented fp32 lut row: 128 v + 1 freq + 63 pad
CH = 512            # idxs per dma_gather call (HW in-flight desc cap)
NHT = H // 128      # 16 h-tiles


def build(b_c):
    jb_n = b_c // 128            # batch blocks per core
    nchunk = b_c * K // CH       # gather chunks
    nslot = CH // 128            # 8 rows per partition per chunk
    tot = b_c * K // 128         # scalar feature columns
    idx_cols = b_c * K // 16

    nc = bacc.Bacc("TRN2", target_bir_lowering=False, debug=False,
                   num_devices=NCORES)

    u_d = nc.dram_tensor("u", [b_c, D], F32, kind="ExternalInput")
    idx_d = nc.dram_tensor("idx", [128, idx_cols], I16, kind="ExternalInput")
    luta_d = nc.dram_tensor("lut_aug", [V, AUG], F32, kind="ExternalInput")
    lutb_d = nc.dram_tensor("lut_bf", [V, D], BF16, kind="ExternalInput")
    w1r_d = nc.dram_tensor("w1r", [NHT, 128, 3, 128], F32R, kind="ExternalInput")
    w1v_d = nc.dram_tensor("w1v", [NHT, 128, K, 128], BF16, kind="ExternalInput")
    wh1_d = nc.dram_tensor("wh1", [NHT, 128, 16, 128], F32R, kind="ExternalInput")
    wh2_d = nc.dram_tensor("wh2", [NHT, 128, 16, 128], F32R, kind="ExternalInput")
    w2_d = nc.dram_tensor("w2", [128, 16, 64], F32R, kind="ExternalInput")
    b1_d = nc.dram_tensor("b1", [128, NHT], F32, kind="ExternalInput")
    bh1_d = nc.dram_tensor("bh1", [128, NHT], F32, kind="ExternalInput")
    bh2_d = nc.dram_tensor("bh2", [128, NHT], F32, kind="ExternalInput")
    b2_d = nc.dram_tensor("b2", [64, 1], F32, kind="ExternalInput")
    t_d = nc.dram_tensor("tT", [64, b_c], F32, kind="ExternalInput")
    id_d = nc.dram_tensor("ident", [128, 128], F32, kind="ExternalInput")

    dist_d = nc.dram_tensor("dist", [64, b_c], F32, kind="ExternalOutput")
    lossp_d = nc.dram_tensor("lossp", [64, 1], F32, kind="ExternalOutput")

    with tile.TileContext(nc) as tc, ExitStack() as ctx:
        pers = ctx.enter_context(tc.tile_pool(name="pers", bufs=1))
        tpp = ctx.enter_context(tc.tile_pool(name="tpp", bufs=2, space="PSUM"))
        hp = ctx.enter_context(tc.tile_pool(name="hp", bufs=2))

        vT = pers.tile([128, nchunk * CH], BF16)
        u_b = pers.tile([128, jb_n, 128], F32)
        uT = pers.tile([128, jb_n * 128], F32R)
        squ = pers.tile([128, jb_n], F32)
        den_u = pers.tile([128, jb_n], F32)
        x_all = pers.tile([128, tot], F32)
        dot_all = pers.tile([128, tot], F32)
        sqv_all = pers.tile([128, tot], F32)
        freq_all = pers.tile([128, tot], F32)
        sc1 = pers.tile([128, tot], F32)
        sc2 = pers.tile([128, tot], F32)
        ext = pers.tile([128, jb_n, 256], F32)
        extT = pers.tile([128, 2, b_c], F32R)
        ident = pers.tile([128, 128], F32)
        b1s = pers.tile([128, NHT], F32)
        bh1s = pers.tile([128, NHT], F32)
        bh2s = pers.tile([128, NHT], F32)
        b2s = pers.tile([64, 1], F32)
        tTs = pers.tile([64, b_c], F32)
        w2s = pers.tile([128, 16, 64], F32R)
        zs = pers.tile([64, b_c], F32)
        ds = pers.tile([64, b_c], F32)
        rz = pers.tile([64, b_c], F32)
        az = pers.tile([64, b_c], F32)
        sp = pers.tile([64, b_c], F32)
        zt = pers.tile([64, b_c], F32)
        lp = pers.tile([64, 1], F32)
        usq = pers.tile([128, 128], F32)
        cm1 = pers.tile([128, 1], F32)
        c1 = pers.tile([128, 1], F32)
        nc.vector.memset(cm1[:], -1.0)
        nc.vector.memset(c1[:], 1.0)

        # phase 0: static loads
        nc.sync.dma_start(u_b[:], u_d.ap().rearrange("(j p) d -> p j d", p=128))
        nc.sync.dma_start(ident[:], id_d.ap())
        nc.sync.dma_start(b1s[:], b1_d.ap())
        nc.sync.dma_start(bh1s[:], bh1_d.ap())
        nc.sync.dma_start(bh2s[:], bh2_d.ap())
        nc.sync.dma_start(b2s[:], b2_d.ap())
        nc.sync.dma_start(tTs[:], t_d.ap())
        nc.sync.dma_start(w2s[:], w2_d.ap())

        # uT = u^T (feature-major), squ = rowsum(u*u), den_u = 1/(1-squ)
        for j in range(jb_n):
            tp = tpp.tile([128, 128], F32, tag="tp")
            nc.tensor.transpose(tp[:], u_b[:, j, :], ident[:])
            nc.vector.tensor_copy(uT[:, j * 128:(j + 1) * 128], tp[:])
            nc.scalar.activation(usq[:], u_b[:, j, :], AF.Square,
                                 accum_out=squ[:, j:j + 1])
        nc.vector.tensor_scalar(den_u[:], squ[:], -1.0, 1.0, ALU.mult, ALU.add)
        nc.vector.reciprocal(den_u[:], den_u[:])

        # phase 1: gathers + per-chunk feature reductions
        with ExitStack() as ctx2:
            idxp = ctx2.enter_context(tc.tile_pool(name="idxp", bufs=1))
            gp = ctx2.enter_context(tc.tile_pool(name="gp", bufs=2))
            idx_sb = idxp.tile([128, idx_cols], I16)
            nc.sync.dma_start(idx_sb[:], idx_d.ap())
            for cc in range(nchunk):
                isl = idx_sb[:, cc * (CH // 16):(cc + 1) * (CH // 16)]
                v_c = gp.tile([128, nslot, AUG], F32, tag="v")
                nc.gpsimd.dma_gather(
                    out_ap=v_c[:], in_ap=luta_d.ap(), idxs_ap=isl,
                    num_idxs=CH, num_idxs_reg=CH, elem_size=AUG)
                nc.gpsimd.dma_gather(
                    out_ap=vT[:, cc * CH:(cc + 1) * CH].rearrange(
                        "p (o x) -> p o x", o=1),
                    in_ap=lutb_d.ap(), idxs_ap=isl,
                    num_idxs=CH, num_idxs_reg=CH, elem_size=D, transpose=True)
                csl = slice(cc * nslot, (cc + 1) * nslot)
                uv = gp.tile([128, nslot, 128], F32, tag="uv")
                u_bc = u_b[:].unsqueeze(1).broadcast_to(
                    [128, nslot // jb_n, jb_n, 128])
                nc.vector.tensor_tensor(
                    uv[:].rearrange("p (a b) d -> p a b d", b=jb_n),
                    v_c[:, :, 0:D].rearrange("p (a b) d -> p a b d", b=jb_n),
                    u_bc, ALU.mult)
                nc.vector.reduce_sum(dot_all[:, csl], uv[:], axis=AX.X)
                vsq = gp.tile([128, nslot, 128], F32, tag="vsq")
                nc.scalar.activation(vsq[:], v_c[:, :, 0:D], AF.Square)
                nc.vector.reduce_sum(sqv_all[:, csl], vsq[:], axis=AX.X)
                nc.vector.tensor_copy(
                    freq_all[:, csl],
                    v_c[:, :, D:D + 1].rearrange("p s o -> p (s o)"))

        # batched per-(b,k) scalar math; scalar col c = k*jb_n + jb
        def bcast(t):  # [128, jb_n] -> [128, 64, jb_n] broadcast view
            return t[:].unsqueeze(1).broadcast_to([128, 64, jb_n])

        def view3(t):  # [128, tot] -> [128, 64, jb_n]
            return t[:].rearrange("p (a b) -> p a b", b=jb_n)

        # sqdist = squ + sqv - 2*dot   (into x_all)
        nc.vector.tensor_scalar(x_all[:], dot_all[:], -2.0, None, ALU.mult)
        nc.vector.tensor_add(x_all[:], x_all[:], sqv_all[:])
        nc.vector.tensor_tensor(view3(x_all), view3(x_all), bcast(squ), ALU.add)
        # den_v = 1/(1-sqv)
        nc.vector.tensor_scalar(sc1[:], sqv_all[:], -1.0, 1.0, ALU.mult, ALU.add)
        nc.vector.reciprocal(sc2[:], sc1[:])
        # x = max(1 + 2*sqdist*den_u*den_v, 1+EPS)
        nc.vector.tensor_mul(x_all[:], x_all[:], sc2[:])
        nc.vector.tensor_tensor(view3(x_all), view3(x_all), bcast(den_u), ALU.mult)
        nc.vector.tensor_scalar(x_all[:], x_all[:], 2.0, 1.0, ALU.mult, ALU.add)
        nc.vector.tensor_scalar(x_all[:], x_all[:], 1.0 + EPS, None, ALU.max)

        # ext views: ext[p, jb, g*64 + k]; source col order is (k outer, jb inner)
        ext_r = ext[:].rearrange("p j f -> p f j")

        def ext_view(g):
            return ext_r[:, g * 64:(g + 1) * 64, :]

        # poincare = log(x + sqrt(x^2 - 1))
        nc.vector.tensor_mul(sc1[:], x_all[:], x_all[:])
        nc.scalar.activation(sc1[:], sc1[:], AF.Sqrt, bias=cm1[:])
        nc.vector.tensor_add(sc1[:], sc1[:], x_all[:])
        nc.scalar.activation(ext_view(0),
                             sc1[:].rearrange("p (a b) -> p a b", b=jb_n),
                             AF.Ln)
        # cos = dot / max(sqrt(squ*sqv), 1e-8)
        nc.vector.tensor_tensor(view3(sc1), view3(sqv_all), bcast(squ), ALU.mult)
        nc.scalar.activation(sc1[:], sc1[:], AF.Sqrt)
        nc.vector.tensor_scalar(sc1[:], sc1[:], 1e-8, None, ALU.max)
        nc.vector.reciprocal(sc2[:], sc1[:])
        nc.vector.tensor_mul(sc2[:], sc2[:], dot_all[:])
        nc.vector.tensor_copy(ext_view(1), view3(sc2))
        # polarization+dot (weights combined on host)
        nc.vector.tensor_copy(ext_view(2), view3(dot_all))
        # popularity = log1p(freq)
        nc.scalar.activation(sc1[:], freq_all[:], AF.Ln, bias=c1[:])
        nc.vector.tensor_copy(ext_view(3), view3(sc1))

        # ext -> extT (feature-major)
        for j in range(jb_n):
            for ft in range(2):
                tp = tpp.tile([128, 128], F32, tag="tp")
                nc.tensor.transpose(tp[:], ext[:, j, ft * 128:(ft + 1) * 128],
                                    ident[:])
                nc.vector.tensor_copy(
                    extT[:, ft, j * 128:(j + 1) * 128], tp[:])

        # phase 2: MLP
        with ExitStack() as ctx3:
            wp = ctx3.enter_context(tc.tile_pool(name="wp", bufs=2))
            psl = ctx3.enter_context(tc.tile_pool(name="psl", bufs=4,
                                                  space="PSUM"))
            psz = ctx3.enter_context(tc.tile_pool(name="psz", bufs=1,
                                                  space="PSUM"))
            h1 = hp.tile([128, NHT, b_c], F32R, tag="h")
            for hh in range(NHT):
                w1v_t = wp.tile([128, K, 128], BF16, tag="w")
                nc.sync.dma_start(w1v_t[:], w1v_d.ap()[hh])
                w1r_t = wp.tile([128, 3, 128], F32R, tag="wr")
                nc.sync.dma_start(w1r_t[:], w1r_d.ap()[hh])
                ps = psl.tile([128, b_c], F32, tag="ps")
                for k in range(K):
                    nc.tensor.matmul(ps[:], w1v_t[:, k, :],
                                     vT[:, k * b_c:(k + 1) * b_c],
                                     start=(k == 0), stop=False)
                nc.tensor.matmul(ps[:], w1r_t[:, 0, :], uT[:],
                                 start=False, stop=False)
                nc.tensor.matmul(ps[:], w1r_t[:, 1, :], extT[:, 0, :],
                                 start=False, stop=False)
                nc.tensor.matmul(ps[:], w1r_t[:, 2, :], extT[:, 1, :],
                                 start=False, stop=True)
                nc.scalar.activation(h1[:, hh, :], ps[:], AF.Relu,
                                     bias=b1s[:, hh:hh + 1])
            hin = h1
            for li, (wd, bs) in enumerate(((wh1_d, bh1s), (wh2_d, bh2s))):
                hout = hp.tile([128, NHT, b_c], F32R, tag="h")
                for hh in range(NHT):
                    wh_t = wp.tile([128, 16, 128], F32R, tag="w")
                    nc.sync.dma_start(wh_t[:], wd.ap()[hh])
                    ps = psl.tile([128, b_c], F32, tag="ps")
                    for kt in range(16):
                        nc.tensor.matmul(ps[:], wh_t[:, kt, :], hin[:, kt, :],
                                         start=(kt == 0), stop=(kt == 15))
                    nc.scalar.activation(hout[:, hh, :], ps[:], AF.Relu,
                                         bias=bs[:, hh:hh + 1])
                hin = hout
            pz = psz.tile([64, b_c], F32)
            for kt in range(16):
                nc.tensor.matmul(pz[:], w2s[:, kt, :], hin[:, kt, :],
                                 start=(kt == 0), stop=(kt == 15))
            # z = logits + b2; distribution = sigmoid(z)
            nc.scalar.activation(zs[:], pz[:], AF.Identity, bias=b2s[:, 0:1])
            nc.scalar.activation(ds[:], zs[:], AF.Sigmoid)
            nc.sync.dma_start(dist_d.ap(), ds[:])
            # loss elems = relu(z) - z*t + softplus(-|z|)
            nc.vector.tensor_scalar(rz[:], zs[:], 0.0, None, ALU.max)
            nc.vector.tensor_scalar(az[:], zs[:], -1.0, None, ALU.mult)
            nc.vector.tensor_tensor(az[:], az[:], zs[:], ALU.max)
            nc.scalar.activation(sp[:], az[:], AF.Exp, scale=-1.0)
            nc.scalar.activation(sp[:], sp[:], AF.Ln, bias=c1[0:64, :])
            nc.vector.tensor_mul(zt[:], zs[:], tTs[:])
            nc.vector.tensor_sub(rz[:], rz[:], zt[:])
            nc.vector.tensor_add(rz[:], rz[:], sp[:])
            nc.vector.reduce_sum(lp[:], rz[:], axis=AX.X)
            nc.sync.dma_start(lossp_d.ap(), lp[:])

    nc.compile()
    return nc


_NC_CACHE = {}


def _get_nc(b_c):
    if b_c not in _NC_CACHE:
        _NC_CACHE[b_c] = build(b_c)
    return _NC_CACHE[b_c]


def _prep_shared(type_lut, frequencies, W1, b1, Wh1, bh1, Wh2, bh2, W2, b2):
    f32 = np.float32
    lut_aug = np.zeros((V, AUG), f32)
    lut_aug[:, :D] = type_lut
    lut_aug[:, D] = frequencies
    lut_bf = type_lut.astype(ml_dtypes.bfloat16)

    # W1 row grouping. neighbor_rep per k occupies rows 128+133k .. 128+133k+132
    # within W1: [v(128), poincare, cos, polar, dot, pop].
    base = D + 133 * np.arange(K)
    w1u = W1[:D]                                  # [128, H]
    poin = W1[base + D]                           # [64, H]
    cosr = W1[base + D + 1]
    pd = W1[base + D + 2] + W1[base + D + 3]      # polar + dot combined
    pop = W1[base + D + 4]
    w1r_rows = np.concatenate([w1u, poin, cosr, pd, pop], 0)   # [384, H]
    w1r = np.ascontiguousarray(
        w1r_rows.reshape(3, 128, NHT, 128).transpose(2, 1, 0, 3)).astype(f32)
    vrows = (D + 133 * np.arange(K)[:, None] + np.arange(D)[None, :]).reshape(-1)
    w1v_rows = W1[vrows]                          # [8192, H]
    w1v = np.ascontiguousarray(
        w1v_rows.reshape(K, 128, NHT, 128).transpose(2, 1, 0, 3)
    ).astype(ml_dtypes.bfloat16)
    wh1 = np.ascontiguousarray(
        Wh1.reshape(16, 128, NHT, 128).transpose(2, 1, 0, 3)).astype(f32)
    wh2 = np.ascontiguousarray(
        Wh2.reshape(16, 128, NHT, 128).transpose(2, 1, 0, 3)).astype(f32)
    w2 = np.ascontiguousarray(
        W2.reshape(16, 128, 64).transpose(1, 0, 2)).astype(f32)
    b1v = np.ascontiguousarray(b1.reshape(NHT, 128).T).astype(f32)
    bh1v = np.ascontiguousarray(bh1.reshape(NHT, 128).T).astype(f32)
    bh2v = np.ascontiguousarray(bh2.reshape(NHT, 128).T).astype(f32)
    b2v = np.ascontiguousarray(b2.reshape(64, 1)).astype(f32)
    ident = np.eye(128, dtype=f32)
    return dict(lut_aug=lut_aug, lut_bf=lut_bf, w1r=w1r, w1v=w1v, wh1=wh1,
                wh2=wh2, w2=w2, b1=b1v, bh1=bh1v, bh2=bh2v, b2=b2v,
                ident=ident)


def kernel(type_embeddings, neighbor_indexes, one_hot_neighbor_types,
           frequencies, type_lut, W1, b1, Wh1, bh1, Wh2, bh2, W2, b2):
    type_embeddings = np.asarray(type_embeddings, np.float32)
    neighbor_indexes = np.asarray(neighbor_indexes)
    one_hot_neighbor_types = np.asarray(one_hot_neighbor_types, np.float32)
    frequencies = np.asarray(frequencies, np.float32)
    type_lut = np.asarray(type_lut, np.float32)
    W1 = np.asarray(W1, np.float32)
    b_c = B // NCORES
    shared = _prep_shared(type_lut, frequencies, W1, np.asarray(b1),
                          np.asarray(Wh1), np.asarray(bh1), np.asarray(Wh2),
                          np.asarray(bh2), np.asarray(W2), np.asarray(b2))
    nc = _get_nc(b_c)
    in_maps = []
    for c in range(NCORES):
        rows = slice(c * b_c, (c + 1) * b_c)
        idx_flat = neighbor_indexes[rows].T.reshape(-1).astype(np.int16)
        idx_w = np.tile(idx_flat.reshape(-1, 16).T, (8, 1)).copy()
        in_maps.append(dict(
            u=type_embeddings[rows],
            idx=idx_w,
            tT=np.ascontiguousarray(one_hot_neighbor_types[rows].T),
            **shared))
    res = run_bass_kernel_spmd(nc, in_maps, core_ids=list(range(NCORES)))
    dist = np.empty((B, K), np.float32)
    total = 0.0
    for c in range(NCORES):
        dist[c * b_c:(c + 1) * b_c] = np.asarray(res.results[c]["dist"]).T
        total += float(np.asarray(res.results[c]["lossp"]).sum())
    loss = np.float32(total / (B * K))
    return dist, loss
